# revision 1
# baseline (speedup 1.0000x reference)
"""Chamfer distance kernel for 8 Trainium2 NeuronCores — v6.

TensorE: 4-band row-tiled matmuls (tile_position=(32r,0), K padded 13->32,
operands replicated at partition offsets 0/32/64/96) — 4 concurrent matmuls
per superstep, full-array activity keeps the HAM clock warm.

Reduction per qtile (4 PSUM groups of 2048 fp32):
  - alternating 1 or 2 groups -> VectorE fused tensor_reduce straight from
    PSUM; the rest -> ScalarE fp32->bf16 copies
  - GpSimdE does the bf16 pairwise merges (frees VectorE)
  - VectorE halving cascade 2048->256 per qtile; batched tail once at the end
"""

import numpy as np
import ml_dtypes

bf16 = ml_dtypes.bfloat16

B = 4
N = 8192            # points per cloud
NQ = N // 2         # queries per core per pass
K = 13              # real contraction rows (padded to 32 per band)
KP = 32
QT = NQ // 128      # query tiles per pass (32)
NCHUNK = 512        # db points per matmul (one PSUM bank fp32)
GROUP = 4           # chunks per PSUM group tile
NGROUP = N // (NCHUNK * GROUP)  # 4 groups per qtile
N_CORES = 8
TREE_OUT = 256
GPSIMD_MERGE = False


def build_bass():
    import concourse.bacc as bacc
    import concourse.mybir as mybir
    from concourse.tile import TileContext

    fp32 = mybir.dt.float32
    bfl6 = mybir.dt.bfloat16
    A = mybir.AluOpType
    AX = mybir.AxisListType
    ACTF = mybir.ActivationFunctionType

    nc = bacc.Bacc()

    la = nc.declare_dram_parameter("la", [128, NQ], bfl6, isOutput=False)
    ra = nc.declare_dram_parameter("ra", [128, N], bfl6, isOutput=False)
    lb = nc.declare_dram_parameter("lb", [128, NQ], bfl6, isOutput=False)
    rb = nc.declare_dram_parameter("rb", [128, N], bfl6, isOutput=False)
    out = nc.declare_dram_parameter("out", [128, 1], fp32, isOutput=True)

    NQT = 2 * QT

    with TileContext(nc) as tc:
        with (
            tc.tile_pool(name="ops", bufs=1) as ops_pool,
            tc.tile_pool(name="psum", bufs=2, space="PSUM") as psum_pool,
            tc.tile_pool(name="exit", bufs=10) as exit_pool,
            tc.tile_pool(name="mrg", bufs=4) as mrg_pool,
        ):
            L = [ops_pool.tile([128, NQ], bfl6, tag="L0", name="L0"),
                 ops_pool.tile([128, NQ], bfl6, tag="L1", name="L1")]
            R = [ops_pool.tile([128, N], bfl6, tag="R0", name="R0"),
                 ops_pool.tile([128, N], bfl6, tag="R1", name="R1")]
            pmw = ops_pool.tile([128, NQT], fp32, tag="pmw")
            W = ops_pool.tile([128, NQT * TREE_OUT], bfl6, tag="W")
            qmin = ops_pool.tile([128, NQT], fp32, tag="qmin")
            acc = ops_pool.tile([128, 1], fp32, tag="acc")

            nc.sync.dma_start(out=L[0][:, :], in_=la[:, :])
            nc.sync.dma_start(out=R[0][:, :], in_=ra[:, :])
            nc.sync.dma_start(out=L[1][:, :], in_=lb[:, :])
            nc.sync.dma_start(out=R[1][:, :], in_=rb[:, :])
            for bp in (0, 32, 64, 96):
                nc.tensor.ldweights(L[0][bp:bp + KP, 0:128],
                                    tile_position=(bp, 0))
                nc.tensor.ldweights(R[0][bp:bp + KP, 0:128],
                                    tile_position=(bp, 0))
                nc.tensor.ldweights(L[1][bp:bp + KP, 0:128],
                                    tile_position=(bp, 0))
                nc.tensor.ldweights(R[1][bp:bp + KP, 0:128],
                                    tile_position=(bp, 0))

            GSZ = GROUP * NCHUNK  # 2048
            for p in range(2):
                for t in range(QT):
                    qi = p * QT + t
                    ndirect = 1
                    es = []
                    for g in range(NGROUP):
                        pg = psum_pool.tile([128, GSZ], fp32, tag="pg")
                        for band in range(GROUP):
                            bp = 32 * band
                            j = (g * GROUP + band) * NCHUNK
                            nc.tensor.matmul(
                                pg[:, band * NCHUNK:(band + 1) * NCHUNK],
                                L[p][bp:bp + KP, t * 128:(t + 1) * 128],
                                R[p][bp:bp + KP, j:j + NCHUNK],
                                start=True, stop=True,
                                tile_position=(bp, 0),
                            )
                        if g < ndirect:
                            nc.vector.tensor_reduce(
                                out=pmw[:, qi:qi + 1],
                                in_=pg[:, :], axis=AX.X, op=A.min,
                            )
                        else:
                            e = exit_pool.tile([128, GSZ], bfl6, tag="e")
                            nc.scalar.activation(e[:, :], pg[:, :], ACTF.Copy)
                            es.append(e)
                    # merge exits to one bf16 group
                    m = mrg_pool.tile([128, GSZ], bfl6, tag="m")
                    eng = nc.gpsimd if GPSIMD_MERGE else nc.vector
                    if len(es) == 3:
                        eng.tensor_tensor(out=m[:, :], in0=es[0][:, :],
                                          in1=es[1][:, :], op=A.min)
                        eng.tensor_tensor(out=m[:, :], in0=m[:, :],
                                          in1=es[2][:, :], op=A.min)
                    else:
                        eng.tensor_tensor(out=m[:, :], in0=es[0][:, :],
                                          in1=es[1][:, :], op=A.min)
                    # halving cascade 2048 -> 256 on VectorE
                    w = 1024
                    while w >= TREE_OUT:
                        dst = (W[:, qi * TREE_OUT:(qi + 1) * TREE_OUT]
                               if w == TREE_OUT else m[:, 0:w])
                        nc.vector.tensor_tensor(
                            out=dst, in0=m[:, 0:w], in1=m[:, w:2 * w], op=A.min)
                        w //= 2
            # batched tail: all qtiles' [256] blocks -> [1]
            Wv = W.rearrange("p (q n) -> p q n", q=NQT)
            w = TREE_OUT // 2
            while w >= 1:
                nc.vector.tensor_tensor(
                    out=Wv[:, :, 0:w], in0=Wv[:, :, 0:w],
                    in1=Wv[:, :, w:2 * w], op=A.min)
                w //= 2
            # min(direct group, tree) per qtile, clamp, sum
            nc.vector.tensor_tensor(out=qmin[:, :], in0=pmw[:, :],
                                    in1=Wv[:, :, 0], op=A.min)
            nc.vector.tensor_scalar(out=qmin[:, :], in0=qmin[:, :],
                                    scalar1=0.0, scalar2=None, op0=A.max)
            nc.vector.tensor_reduce(out=acc[:, :], in_=qmin[:, :],
                                    axis=AX.X, op=A.add)
            nc.sync.dma_start(out=out[:, :], in_=acc[:, :])
    nc.finalize()
    return nc


def _split_bf16(x):
    hi = x.astype(bf16)
    lo = (x - hi.astype(np.float32)).astype(bf16)
    return hi, lo


def _pad_bands(rows):
    """[13, n] bf16 -> [128, n]: pad K to 32 with zeros, replicate 4x."""
    n = rows.shape[1]
    k32 = np.zeros((KP, n), dtype=bf16)
    k32[:K] = rows
    return np.concatenate([k32] * 4, axis=0)


def _make_lhsT(q):
    x = np.ascontiguousarray(q.T).astype(np.float32)
    x2 = np.sum(q * q, axis=-1, dtype=np.float32)
    xh, xl = _split_bf16(x)
    x2h, x2l = _split_bf16(x2)
    ones = np.ones_like(x2, dtype=bf16)
    rows = np.concatenate([xh, xh, xl, x2h[None], x2l[None],
                           ones[None], ones[None]], axis=0)
    return _pad_bands(rows)


def _make_rhs(d):
    y = np.ascontiguousarray((-2.0 * d.T)).astype(np.float32)
    y2 = np.sum(d * d, axis=-1, dtype=np.float32)
    yh, yl = _split_bf16(y)
    y2h, y2l = _split_bf16(y2)
    ones = np.ones_like(y2, dtype=bf16)
    rows = np.concatenate([yh, yl, yh, ones[None], ones[None],
                           y2h[None], y2l[None]], axis=0)
    return _pad_bands(rows)


def make_in_maps(points1, points2):
    p1 = np.asarray(points1, dtype=np.float32)
    p2 = np.asarray(points2, dtype=np.float32)
    in_maps = []
    for i in range(N_CORES):
        b, h = divmod(i, 2)
        qa = p1[b, h * NQ:(h + 1) * NQ]
        qb = p2[b, h * NQ:(h + 1) * NQ]
        in_maps.append({
            "la": _make_lhsT(qa), "ra": _make_rhs(p2[b]),
            "lb": _make_lhsT(qb), "rb": _make_rhs(p1[b]),
        })
    return in_maps


_CACHE = {}


def kernel(points1, points2):
    from concourse.bass_utils import run_bass_kernel_spmd

    if "nc" not in _CACHE:
        _CACHE["nc"] = build_bass()
    nc = _CACHE["nc"]
    in_maps = make_in_maps(points1, points2)
    res = run_bass_kernel_spmd(nc, in_maps, core_ids=list(range(N_CORES)))
    total = 0.0
    for i in range(N_CORES):
        total += float(res.results[i]["out"].astype(np.float64).sum())
    return np.float32(total / N)



# revision 8
# speedup vs baseline: 1.0026x; 1.0026x over previous
"""Chamfer distance kernel for 8 Trainium2 NeuronCores — v7 (shared matrix).

Each core computes ONE [4096 x 8192] block of the per-batch distance
matrix (p1-half rows x all p2 cols) instead of two directional passes:
half the matmul + PSUM-drain volume of v6.  Both reductions come from
the same block:
  drain: ScalarE ACT Copy with scale=-1 drains PSUM->SBUF bf16 (e = -d).
  dir1 (per-p1-row min): VectorE pool_max per group -> acc slot
       (max of -d == -min d); single-src pool runs in a fast DVE mode.
  dir2 (per-p2-col min): VectorE running elementwise-max chain over
       qtiles into Racc [128, 8192] (still negated); host max-combines
       the two cores of each batch and reduces over the partition axis
       (the "min-combining" step of the sharding hint).
"""

import os
import numpy as np
import ml_dtypes

bf16 = ml_dtypes.bfloat16

B = 4
N = 8192            # points per cloud
NQ = N // 2         # p1 rows handled per core
K = 13              # real contraction rows (padded to 32 per band)
KP = 32
QT = NQ // 128      # query tiles per core (32)
NCHUNK = 512        # db points per matmul (one PSUM bank stripe)
GROUP = 4           # chunks per PSUM group tile
GSZ = GROUP * NCHUNK            # 2048
NGROUP = N // GSZ   # 4 groups per qtile
N_CORES = 8
BIGF = 1.0e30

DIR1 = os.environ.get("V7_DIR1", "pool")   # pool | cascade
USE_CHAIN = os.environ.get("V7_CHAIN", "1") == "1"


def build_bass():
    import concourse.bacc as bacc
    import concourse.mybir as mybir
    from concourse.tile import TileContext

    fp32 = mybir.dt.float32
    bfl6 = mybir.dt.bfloat16
    A = mybir.AluOpType
    AX = mybir.AxisListType
    ACTF = mybir.ActivationFunctionType

    nc = bacc.Bacc()

    la = nc.declare_dram_parameter("la", [128, NQ], bfl6, isOutput=False)
    ra = nc.declare_dram_parameter("ra", [128, N], bfl6, isOutput=False)
    out1 = nc.declare_dram_parameter("out1", [128, 1], fp32, isOutput=True)
    out2 = nc.declare_dram_parameter("out2", [128, N], bfl6, isOutput=True)

    with TileContext(nc) as tc:
        with (
            tc.tile_pool(name="ops", bufs=1) as ops_pool,
            tc.tile_pool(name="psum", bufs=2, space="PSUM") as psum_pool,
            tc.tile_pool(name="exit", bufs=6) as e_pool,
            tc.tile_pool(name="scrap", bufs=2) as sc_pool,
        ):
            L = ops_pool.tile([128, NQ], bfl6, tag="L", name="L")
            Rf = ops_pool.tile([128, N], bfl6, tag="Rf", name="Rf")
            Racc = ops_pool.tile([128, N], bfl6, tag="Racc", name="Racc")
            acc4 = ops_pool.tile([128, QT * NGROUP], fp32, tag="acc4")
            qmin = ops_pool.tile([128, QT], fp32, tag="qmin")
            accsum = ops_pool.tile([128, 1], fp32, tag="accsum")

            nc.sync.dma_start(out=L[:, :], in_=la[:, :])
            nc.sync.dma_start(out=Rf[:, :], in_=ra[:, :])
            for bp in (0, 32, 64, 96):
                nc.tensor.ldweights(L[bp:bp + KP, 0:128],
                                    tile_position=(bp, 0))
                nc.tensor.ldweights(Rf[bp:bp + KP, 0:128],
                                    tile_position=(bp, 0))

            for t in range(QT):
                for g in range(NGROUP):
                    pg = psum_pool.tile([128, GSZ], fp32, tag="pg")
                    for band in range(GROUP):
                        bp = 32 * band
                        j = (g * GROUP + band) * NCHUNK
                        nc.tensor.matmul(
                            pg[:, band * NCHUNK:(band + 1) * NCHUNK],
                            L[bp:bp + KP, t * 128:(t + 1) * 128],
                            Rf[bp:bp + KP, j:j + NCHUNK],
                            start=True, stop=True,
                            tile_position=(bp, 0),
                        )
                    # e = -d in bf16
                    e = e_pool.tile([128, GSZ], bfl6, tag="e")
                    nc.scalar.activation(e[:, :], pg[:, :], ACTF.Copy,
                                         scale=-1.0)
                    k = t * NGROUP + g
                    if DIR1 == "pool":
                        nc.vector.pool_max(out=acc4[:, k:k + 1], in_=e[:, :])
                    else:
                        sc = sc_pool.tile([128, GSZ // 2], bfl6, tag="sc")
                        nc.vector.tensor_tensor(
                            out=sc[:, :], in0=e[:, 0:GSZ // 2],
                            in1=e[:, GSZ // 2:GSZ], op=A.max)
                        w = GSZ // 4
                        while w >= 1:
                            nc.vector.tensor_tensor(
                                out=sc[:, 0:w], in0=sc[:, 0:w],
                                in1=sc[:, w:2 * w], op=A.max)
                            w //= 2
                        nc.vector.tensor_scalar(
                            out=acc4[:, k:k + 1], in0=sc[:, 0:1],
                            scalar1=0.0, scalar2=None, op0=A.max)
                    if USE_CHAIN:
                        rg = Racc[:, g * GSZ:(g + 1) * GSZ]
                        if t == 0:
                            nc.vector.tensor_scalar(
                                out=rg, in0=e[:, :], scalar1=-BIGF,
                                scalar2=None, op0=A.max)
                        else:
                            nc.vector.tensor_tensor(
                                out=rg, in0=e[:, :], in1=rg, op=A.max)

            if not USE_CHAIN:
                nc.vector.memset(Racc[:, :], 0.0)
            # acc4 holds max(-d) per (qtile, group); combine groups,
            # then dist1 per query = max(-m, 0) = relu(-m)
            acc4v = acc4.rearrange("p (t g) -> p t g", g=NGROUP)
            nc.vector.tensor_tensor(
                out=acc4v[:, :, 0], in0=acc4v[:, :, 0],
                in1=acc4v[:, :, 1], op=A.max)
            nc.vector.tensor_tensor(
                out=acc4v[:, :, 2], in0=acc4v[:, :, 2],
                in1=acc4v[:, :, 3], op=A.max)
            nc.vector.tensor_tensor(
                out=qmin[:, :], in0=acc4v[:, :, 0],
                in1=acc4v[:, :, 2], op=A.max)
            nc.vector.tensor_scalar(
                out=qmin[:, :], in0=qmin[:, :], scalar1=-1.0, scalar2=0.0,
                op0=A.mult, op1=A.max)
            nc.vector.tensor_reduce(out=accsum[:, :], in_=qmin[:, :],
                                    axis=AX.X, op=A.add)
            nc.sync.dma_start(out=out1[:, :], in_=accsum[:, :])
            nc.sync.dma_start(out=out2[:, :], in_=Racc[:, :])
    nc.finalize()
    return nc


def _split_bf16(x):
    hi = x.astype(bf16)
    lo = (x - hi.astype(np.float32)).astype(bf16)
    return hi, lo


def _pad_bands(rows):
    """[13, n] bf16 -> [128, n]: pad K to 32 with zeros, replicate 4x."""
    n = rows.shape[1]
    k32 = np.zeros((KP, n), dtype=bf16)
    k32[:K] = rows
    return np.concatenate([k32] * 4, axis=0)


def _make_lhsT(q):
    x = np.ascontiguousarray(q.T).astype(np.float32)
    x2 = np.sum(q * q, axis=-1, dtype=np.float32)
    xh, xl = _split_bf16(x)
    x2h, x2l = _split_bf16(x2)
    ones = np.ones_like(x2, dtype=bf16)
    rows = np.concatenate([xh, xh, xl, x2h[None], x2l[None],
                           ones[None], ones[None]], axis=0)
    return _pad_bands(rows)


def _make_rhs(d):
    y = np.ascontiguousarray((-2.0 * d.T)).astype(np.float32)
    y2 = np.sum(d * d, axis=-1, dtype=np.float32)
    yh, yl = _split_bf16(y)
    y2h, y2l = _split_bf16(y2)
    ones = np.ones_like(y2, dtype=bf16)
    rows = np.concatenate([yh, yl, yh, ones[None], ones[None],
                           y2h[None], y2l[None]], axis=0)
    return _pad_bands(rows)


def make_in_maps(points1, points2):
    p1 = np.asarray(points1, dtype=np.float32)
    p2 = np.asarray(points2, dtype=np.float32)
    in_maps = []
    for i in range(N_CORES):
        b, h = divmod(i, 2)
        in_maps.append({
            "la": _make_lhsT(p1[b, h * NQ:(h + 1) * NQ]),
            "ra": _make_rhs(p2[b]),
        })
    return in_maps


def combine(results):
    """Host-side gather: dir1 sums + max-combine of negated dir2 partials."""
    total = 0.0
    for i in range(N_CORES):
        total += float(results[i]["out1"].astype(np.float64).sum())
    for b in range(B):
        r = np.maximum(results[2 * b]["out2"], results[2 * b + 1]["out2"])
        colmax = r.astype(np.float32).max(axis=0)
        total += float(np.maximum(-colmax, 0.0).astype(np.float64).sum())
    return np.float32(total / N)


_CACHE = {}


def kernel(points1, points2):
    from concourse.bass_utils import run_bass_kernel_spmd

    if "nc" not in _CACHE:
        _CACHE["nc"] = build_bass()
    nc = _CACHE["nc"]
    in_maps = make_in_maps(points1, points2)
    res = run_bass_kernel_spmd(nc, in_maps, core_ids=list(range(N_CORES)))
    return combine(res.results)


# revision 11
# speedup vs baseline: 1.3705x; 1.3669x over previous
"""Chamfer distance kernel for 8 Trainium2 NeuronCores — v7 (shared matrix).

Each core computes ONE [4096 x 8192] block of the per-batch distance
matrix (p1-half rows x all p2 cols) instead of two directional passes:
half the matmul + PSUM-drain volume of v6.  Both reductions come from
the same block:
  drain: ScalarE ACT Copy with scale=-1 drains PSUM->SBUF bf16 (e = -d).
  dir1 (per-p1-row min): VectorE pool_max per group -> acc slot
       (max of -d == -min d); single-src pool runs in a fast DVE mode.
  dir2 (per-p2-col min): VectorE running elementwise-max chain over
       qtiles into Racc [128, 8192] (still negated); host max-combines
       the two cores of each batch and reduces over the partition axis
       (the "min-combining" step of the sharding hint).
"""

import os
import numpy as np
import ml_dtypes

bf16 = ml_dtypes.bfloat16

B = 4
N = 8192            # points per cloud
NQ = N // 2         # p1 rows handled per core
K = 13              # real contraction rows (padded to 32 per band)
KP = 32
QT = NQ // 128      # query tiles per core (32)
NCHUNK = 512        # db points per matmul (one PSUM bank stripe)
GROUP = 4           # chunks per PSUM group tile
GSZ = GROUP * NCHUNK            # 2048
NGROUP = N // GSZ   # 4 groups per qtile
N_CORES = 8
BIGF = 1.0e30

GCHAIN = int(os.environ.get("V7_GCHAIN", "0"))  # groups chained on gpsimd
TREE_OUT = 64       # per-qtile dir1 tree stops at this width


def build_bass():
    import concourse.bacc as bacc
    import concourse.mybir as mybir
    from concourse.tile import TileContext

    fp32 = mybir.dt.float32
    bfl6 = mybir.dt.bfloat16
    A = mybir.AluOpType
    AX = mybir.AxisListType
    ACTF = mybir.ActivationFunctionType

    nc = bacc.Bacc()

    la = nc.declare_dram_parameter("la", [128, NQ], bfl6, isOutput=False)
    ra = nc.declare_dram_parameter("ra", [128, N], bfl6, isOutput=False)
    out1 = nc.declare_dram_parameter("out1", [128, 1], fp32, isOutput=True)
    out2 = nc.declare_dram_parameter("out2", [128, N], bfl6, isOutput=True)

    with TileContext(nc) as tc:
        with (
            tc.tile_pool(name="ops", bufs=1) as ops_pool,
            tc.tile_pool(name="psum", bufs=2, space="PSUM") as psum_pool,
            tc.tile_pool(name="exit", bufs=6) as e_pool,
            tc.tile_pool(name="scrap", bufs=2) as sc_pool,
        ):
            L = ops_pool.tile([128, NQ], bfl6, tag="L", name="L")
            Rf = ops_pool.tile([128, N], bfl6, tag="Rf", name="Rf")
            Racc = ops_pool.tile([128, N], bfl6, tag="Racc", name="Racc")
            W = ops_pool.tile([128, QT * TREE_OUT], bfl6, tag="W")
            qmin = ops_pool.tile([128, QT], fp32, tag="qmin")
            accsum = ops_pool.tile([128, 1], fp32, tag="accsum")

            nc.sync.dma_start(out=L[:, :], in_=la[:, :])
            nc.sync.dma_start(out=Rf[:, :], in_=ra[:, :])
            for bp in (0, 32, 64, 96):
                nc.tensor.ldweights(L[bp:bp + KP, 0:128],
                                    tile_position=(bp, 0))
                nc.tensor.ldweights(Rf[bp:bp + KP, 0:128],
                                    tile_position=(bp, 0))

            for t in range(QT):
                es = []
                M = sc_pool.tile([128, GSZ], bfl6, tag="m")
                for g in range(NGROUP):
                    pg = psum_pool.tile([128, GSZ], fp32, tag="pg")
                    for band in range(GROUP):
                        bp = 32 * band
                        j = (g * GROUP + band) * NCHUNK
                        nc.tensor.matmul(
                            pg[:, band * NCHUNK:(band + 1) * NCHUNK],
                            L[bp:bp + KP, t * 128:(t + 1) * 128],
                            Rf[bp:bp + KP, j:j + NCHUNK],
                            start=True, stop=True,
                            tile_position=(bp, 0),
                        )
                    # e = -d in bf16
                    e = e_pool.tile([128, GSZ], bfl6, tag="e")
                    nc.scalar.activation(e[:, :], pg[:, :], ACTF.Copy,
                                         scale=-1.0)
                    es.append(e)
                    # dir1: elementwise max-fold across groups (same queries)
                    if g == 1:
                        nc.vector.tensor_tensor(
                            out=M[:, :], in0=es[0][:, :], in1=es[1][:, :],
                            op=A.max)
                    elif g > 1:
                        nc.vector.tensor_tensor(
                            out=M[:, :], in0=e[:, :], in1=M[:, :], op=A.max)
                    # dir2: running col-max chain
                    rg = Racc[:, g * GSZ:(g + 1) * GSZ]
                    eng = nc.gpsimd if g < GCHAIN else nc.vector
                    if t == 0:
                        nc.vector.tensor_scalar(
                            out=rg, in0=e[:, :], scalar1=-BIGF,
                            scalar2=None, op0=A.max)
                    else:
                        eng.tensor_tensor(
                            out=rg, in0=e[:, :], in1=rg, op=A.max)
                # dir1 tree: M [2048] -> W[:, t*64:(t+1)*64]
                w = GSZ // 2
                while w >= TREE_OUT:
                    dst = (W[:, t * TREE_OUT:(t + 1) * TREE_OUT]
                           if w == TREE_OUT else M[:, 0:w])
                    nc.vector.tensor_tensor(
                        out=dst, in0=M[:, 0:w], in1=M[:, w:2 * w], op=A.max)
                    w //= 2

            # batched tail: [128, QT, 64] -> [128, QT]
            Wv = W.rearrange("p (t n) -> p t n", t=QT)
            w = TREE_OUT // 2
            while w >= 1:
                nc.vector.tensor_tensor(
                    out=Wv[:, :, 0:w], in0=Wv[:, :, 0:w],
                    in1=Wv[:, :, w:2 * w], op=A.max)
                w //= 2
            # W holds max(-d) per qtile; dist1 per query = max(-m, 0)
            nc.vector.tensor_scalar(
                out=qmin[:, :], in0=Wv[:, :, 0], scalar1=-1.0, scalar2=0.0,
                op0=A.mult, op1=A.max)
            nc.vector.tensor_reduce(out=accsum[:, :], in_=qmin[:, :],
                                    axis=AX.X, op=A.add)
            nc.sync.dma_start(out=out1[:, :], in_=accsum[:, :])
            nc.sync.dma_start(out=out2[:, :], in_=Racc[:, :])
    nc.finalize()
    return nc


def _split_bf16(x):
    hi = x.astype(bf16)
    lo = (x - hi.astype(np.float32)).astype(bf16)
    return hi, lo


def _pad_bands(rows):
    """[13, n] bf16 -> [128, n]: pad K to 32 with zeros, replicate 4x."""
    n = rows.shape[1]
    k32 = np.zeros((KP, n), dtype=bf16)
    k32[:K] = rows
    return np.concatenate([k32] * 4, axis=0)


def _make_lhsT(q):
    x = np.ascontiguousarray(q.T).astype(np.float32)
    x2 = np.sum(q * q, axis=-1, dtype=np.float32)
    xh, xl = _split_bf16(x)
    x2h, x2l = _split_bf16(x2)
    ones = np.ones_like(x2, dtype=bf16)
    rows = np.concatenate([xh, xh, xl, x2h[None], x2l[None],
                           ones[None], ones[None]], axis=0)
    return _pad_bands(rows)


def _make_rhs(d):
    y = np.ascontiguousarray((-2.0 * d.T)).astype(np.float32)
    y2 = np.sum(d * d, axis=-1, dtype=np.float32)
    yh, yl = _split_bf16(y)
    y2h, y2l = _split_bf16(y2)
    ones = np.ones_like(y2, dtype=bf16)
    rows = np.concatenate([yh, yl, yh, ones[None], ones[None],
                           y2h[None], y2l[None]], axis=0)
    return _pad_bands(rows)


def make_in_maps(points1, points2):
    p1 = np.asarray(points1, dtype=np.float32)
    p2 = np.asarray(points2, dtype=np.float32)
    in_maps = []
    for i in range(N_CORES):
        b, h = divmod(i, 2)
        in_maps.append({
            "la": _make_lhsT(p1[b, h * NQ:(h + 1) * NQ]),
            "ra": _make_rhs(p2[b]),
        })
    return in_maps


def combine(results):
    """Host-side gather: dir1 sums + max-combine of negated dir2 partials."""
    total = 0.0
    for i in range(N_CORES):
        total += float(results[i]["out1"].astype(np.float64).sum())
    for b in range(B):
        r = np.maximum(results[2 * b]["out2"], results[2 * b + 1]["out2"])
        colmax = r.astype(np.float32).max(axis=0)
        total += float(np.maximum(-colmax, 0.0).astype(np.float64).sum())
    return np.float32(total / N)


_CACHE = {}


def kernel(points1, points2):
    from concourse.bass_utils import run_bass_kernel_spmd

    if "nc" not in _CACHE:
        _CACHE["nc"] = build_bass()
    nc = _CACHE["nc"]
    in_maps = make_in_maps(points1, points2)
    res = run_bass_kernel_spmd(nc, in_maps, core_ids=list(range(N_CORES)))
    return combine(res.results)


# revision 13
# speedup vs baseline: 1.4235x; 1.0387x over previous
"""Chamfer distance kernel for 8 Trainium2 NeuronCores — v7 (shared matrix).

Each core computes ONE [4096 x 8192] block of the per-batch distance
matrix (p1-half rows x all p2 cols) instead of two directional passes:
half the matmul + PSUM-drain volume of v6.  Both reductions come from
the same block:
  drain: ScalarE ACT Copy with scale=-1 drains PSUM->SBUF bf16 (e = -d).
  dir1 (per-p1-row min): VectorE pool_max per group -> acc slot
       (max of -d == -min d); single-src pool runs in a fast DVE mode.
  dir2 (per-p2-col min): VectorE running elementwise-max chain over
       qtiles into Racc [128, 8192] (still negated); host max-combines
       the two cores of each batch and reduces over the partition axis
       (the "min-combining" step of the sharding hint).
"""

import os
import numpy as np
import ml_dtypes

bf16 = ml_dtypes.bfloat16

B = 4
N = 8192            # points per cloud
NQ = N // 2         # p1 rows handled per core
K = 13              # real contraction rows (padded to 32 per band)
KP = 32
QT = NQ // 128      # query tiles per core (32)
NCHUNK = 512        # db points per matmul (one PSUM bank stripe)
GROUP = 4           # chunks per PSUM group tile
GSZ = GROUP * NCHUNK            # 2048
NGROUP = N // GSZ   # 4 groups per qtile
N_CORES = 8
BIGF = 1.0e30

DMACHAIN = int(os.environ.get("V7_DMACHAIN", "0"))  # groups chained via DMA CCE
TREE_OUT = int(os.environ.get("V7_TREEOUT", "64"))


def build_bass():
    import concourse.bacc as bacc
    import concourse.mybir as mybir
    from concourse.tile import TileContext

    fp32 = mybir.dt.float32
    bfl6 = mybir.dt.bfloat16
    A = mybir.AluOpType
    AX = mybir.AxisListType
    ACTF = mybir.ActivationFunctionType

    nc = bacc.Bacc()

    la = nc.declare_dram_parameter("la", [128, NQ], bfl6, isOutput=False)
    ra = nc.declare_dram_parameter("ra", [128, N], bfl6, isOutput=False)
    out1 = nc.declare_dram_parameter("out1", [128, 1], fp32, isOutput=True)
    out2 = nc.declare_dram_parameter("out2", [128, N], bfl6, isOutput=True)

    with TileContext(nc) as tc:
        with (
            tc.tile_pool(name="ops", bufs=1) as ops_pool,
            tc.tile_pool(name="psum", bufs=2, space="PSUM") as psum_pool,
            tc.tile_pool(name="exit", bufs=6) as e_pool,
            tc.tile_pool(name="scrap", bufs=2) as sc_pool,
        ):
            L = ops_pool.tile([128, NQ], bfl6, tag="L", name="L")
            Rf = ops_pool.tile([128, N], bfl6, tag="Rf", name="Rf")
            Racc = ops_pool.tile([128, N], bfl6, tag="Racc", name="Racc")
            W = ops_pool.tile([128, QT * TREE_OUT], bfl6, tag="W")
            qmin = ops_pool.tile([128, QT], fp32, tag="qmin")
            accsum = ops_pool.tile([128, 1], fp32, tag="accsum")

            nc.sync.dma_start(out=L[:, :], in_=la[:, :])
            nc.sync.dma_start(out=Rf[:, :], in_=ra[:, :])
            for bp in (0, 32, 64, 96):
                nc.tensor.ldweights(L[bp:bp + KP, 0:128],
                                    tile_position=(bp, 0))
                nc.tensor.ldweights(Rf[bp:bp + KP, 0:128],
                                    tile_position=(bp, 0))

            for t in range(QT):
                es = []
                M = sc_pool.tile([128, GSZ], bfl6, tag="m")
                for g in range(NGROUP):
                    pg = psum_pool.tile([128, GSZ], fp32, tag="pg")
                    for band in range(GROUP):
                        bp = 32 * band
                        j = (g * GROUP + band) * NCHUNK
                        nc.tensor.matmul(
                            pg[:, band * NCHUNK:(band + 1) * NCHUNK],
                            L[bp:bp + KP, t * 128:(t + 1) * 128],
                            Rf[bp:bp + KP, j:j + NCHUNK],
                            start=True, stop=True,
                            tile_position=(bp, 0),
                        )
                    # e = -d in bf16
                    e = e_pool.tile([128, GSZ], bfl6, tag="e")
                    nc.scalar.activation(e[:, :], pg[:, :], ACTF.Copy,
                                         scale=-1.0)
                    es.append(e)
                    # dir1: elementwise max-fold across groups (same queries)
                    if g == 1:
                        nc.vector.tensor_tensor(
                            out=M[:, :], in0=es[0][:, :], in1=es[1][:, :],
                            op=A.max)
                    elif g > 1:
                        nc.vector.tensor_tensor(
                            out=M[:, :], in0=e[:, :], in1=M[:, :], op=A.max)
                    # dir2: running col-max chain
                    rg = Racc[:, g * GSZ:(g + 1) * GSZ]
                    if t == 0:
                        nc.vector.tensor_scalar(
                            out=rg, in0=e[:, :], scalar1=-BIGF,
                            scalar2=None, op0=A.max)
                    elif g < DMACHAIN:
                        nc.gpsimd.dma_start(out=rg, in_=e[:, :],
                                            accum_op=A.max)
                    else:
                        nc.vector.tensor_tensor(
                            out=rg, in0=e[:, :], in1=rg, op=A.max)
                # dir1 tree: M [2048] -> W[:, t*64:(t+1)*64]
                w = GSZ // 2
                while w >= TREE_OUT:
                    dst = (W[:, t * TREE_OUT:(t + 1) * TREE_OUT]
                           if w == TREE_OUT else M[:, 0:w])
                    nc.vector.tensor_tensor(
                        out=dst, in0=M[:, 0:w], in1=M[:, w:2 * w], op=A.max)
                    w //= 2

            # batched tail: [128, QT, 64] -> [128, QT]
            Wv = W.rearrange("p (t n) -> p t n", t=QT)
            w = TREE_OUT // 2
            while w >= 1:
                nc.vector.tensor_tensor(
                    out=Wv[:, :, 0:w], in0=Wv[:, :, 0:w],
                    in1=Wv[:, :, w:2 * w], op=A.max)
                w //= 2
            # W holds max(-d) per qtile; dist1 per query = max(-m, 0)
            nc.vector.tensor_scalar(
                out=qmin[:, :], in0=Wv[:, :, 0], scalar1=-1.0, scalar2=0.0,
                op0=A.mult, op1=A.max)
            nc.vector.tensor_reduce(out=accsum[:, :], in_=qmin[:, :],
                                    axis=AX.X, op=A.add)
            nc.sync.dma_start(out=out1[:, :], in_=accsum[:, :])
            nc.sync.dma_start(out=out2[:, :], in_=Racc[:, :])
    nc.finalize()
    return nc


def _split_bf16(x):
    hi = x.astype(bf16)
    lo = (x - hi.astype(np.float32)).astype(bf16)
    return hi, lo


def _pad_bands(rows):
    """[13, n] bf16 -> [128, n]: pad K to 32 with zeros, replicate 4x."""
    n = rows.shape[1]
    k32 = np.zeros((KP, n), dtype=bf16)
    k32[:K] = rows
    return np.concatenate([k32] * 4, axis=0)


def _make_lhsT(q):
    x = np.ascontiguousarray(q.T).astype(np.float32)
    x2 = np.sum(q * q, axis=-1, dtype=np.float32)
    xh, xl = _split_bf16(x)
    x2h, x2l = _split_bf16(x2)
    ones = np.ones_like(x2, dtype=bf16)
    rows = np.concatenate([xh, xh, xl, x2h[None], x2l[None],
                           ones[None], ones[None]], axis=0)
    return _pad_bands(rows)


def _make_rhs(d):
    y = np.ascontiguousarray((-2.0 * d.T)).astype(np.float32)
    y2 = np.sum(d * d, axis=-1, dtype=np.float32)
    yh, yl = _split_bf16(y)
    y2h, y2l = _split_bf16(y2)
    ones = np.ones_like(y2, dtype=bf16)
    rows = np.concatenate([yh, yl, yh, ones[None], ones[None],
                           y2h[None], y2l[None]], axis=0)
    return _pad_bands(rows)


def make_in_maps(points1, points2):
    p1 = np.asarray(points1, dtype=np.float32)
    p2 = np.asarray(points2, dtype=np.float32)
    in_maps = []
    for i in range(N_CORES):
        b, h = divmod(i, 2)
        in_maps.append({
            "la": _make_lhsT(p1[b, h * NQ:(h + 1) * NQ]),
            "ra": _make_rhs(p2[b]),
        })
    return in_maps


def combine(results):
    """Host-side gather: dir1 sums + max-combine of negated dir2 partials."""
    total = 0.0
    for i in range(N_CORES):
        total += float(results[i]["out1"].astype(np.float64).sum())
    for b in range(B):
        r = np.maximum(results[2 * b]["out2"], results[2 * b + 1]["out2"])
        colmax = r.astype(np.float32).max(axis=0)
        total += float(np.maximum(-colmax, 0.0).astype(np.float64).sum())
    return np.float32(total / N)


_CACHE = {}


def kernel(points1, points2):
    from concourse.bass_utils import run_bass_kernel_spmd

    if "nc" not in _CACHE:
        _CACHE["nc"] = build_bass()
    nc = _CACHE["nc"]
    in_maps = make_in_maps(points1, points2)
    res = run_bass_kernel_spmd(nc, in_maps, core_ids=list(range(N_CORES)))
    return combine(res.results)


# revision 16
# speedup vs baseline: 1.4378x; 1.0101x over previous
"""Chamfer distance kernel for 8 Trainium2 NeuronCores — v7 (shared matrix).

Each core computes ONE [4096 x 8192] block of the per-batch distance
matrix (p1-half rows x all p2 cols) instead of two directional passes:
half the matmul + PSUM-drain volume of v6.  Both reductions come from
the same block:
  drain: ScalarE ACT Copy with scale=-1 drains PSUM->SBUF bf16 (e = -d).
  dir1 (per-p1-row min): VectorE pool_max per group -> acc slot
       (max of -d == -min d); single-src pool runs in a fast DVE mode.
  dir2 (per-p2-col min): VectorE running elementwise-max chain over
       qtiles into Racc [128, 8192] (still negated); host max-combines
       the two cores of each batch and reduces over the partition axis
       (the "min-combining" step of the sharding hint).
"""

import os
import numpy as np
import ml_dtypes

bf16 = ml_dtypes.bfloat16

B = 4
N = 8192            # points per cloud
NQ = N // 2         # p1 rows handled per core
K = 13              # real contraction rows (padded to 32 per band)
KP = 32
QT = NQ // 128      # query tiles per core (32)
NCHUNK = 512        # db points per matmul (one PSUM bank stripe)
GROUP = 4           # chunks per PSUM group tile
GSZ = GROUP * NCHUNK            # 2048
NGROUP = N // GSZ   # 4 groups per qtile
N_CORES = 8
BIGF = 1.0e30

DMACHAIN = int(os.environ.get("V7_DMACHAIN", "0"))  # groups chained via DMA CCE
TREE_OUT = int(os.environ.get("V7_TREEOUT", "1024"))


def build_bass():
    import concourse.bacc as bacc
    import concourse.mybir as mybir
    from concourse.tile import TileContext

    fp32 = mybir.dt.float32
    bfl6 = mybir.dt.bfloat16
    A = mybir.AluOpType
    AX = mybir.AxisListType
    ACTF = mybir.ActivationFunctionType

    nc = bacc.Bacc()

    la = nc.declare_dram_parameter("la", [128, NQ], bfl6, isOutput=False)
    ra = nc.declare_dram_parameter("ra", [128, N], bfl6, isOutput=False)
    out1 = nc.declare_dram_parameter("out1", [128, 1], fp32, isOutput=True)
    out2 = nc.declare_dram_parameter("out2", [128, N], bfl6, isOutput=True)

    with TileContext(nc) as tc:
        with (
            tc.tile_pool(name="ops", bufs=1) as ops_pool,
            tc.tile_pool(name="psum", bufs=2, space="PSUM") as psum_pool,
            tc.tile_pool(name="exit", bufs=6) as e_pool,
            tc.tile_pool(name="scrap", bufs=2) as sc_pool,
        ):
            L = ops_pool.tile([128, NQ], bfl6, tag="L", name="L")
            Rf = ops_pool.tile([128, N], bfl6, tag="Rf", name="Rf")
            Racc = ops_pool.tile([128, N], bfl6, tag="Racc", name="Racc")
            W = ops_pool.tile([128, QT * TREE_OUT], bfl6, tag="W")
            qmin = ops_pool.tile([128, QT], fp32, tag="qmin")
            accsum = ops_pool.tile([128, 1], fp32, tag="accsum")

            nc.scalar.dma_start(out=L[:, :], in_=la[:, :])
            for c in range(NGROUP):
                nc.sync.dma_start(out=Rf[:, c * GSZ:(c + 1) * GSZ],
                                  in_=ra[:, c * GSZ:(c + 1) * GSZ])
            for bp in (0, 32, 64, 96):
                nc.tensor.ldweights(L[bp:bp + KP, 0:128],
                                    tile_position=(bp, 0))
                nc.tensor.ldweights(Rf[bp:bp + KP, 0:128],
                                    tile_position=(bp, 0))

            for t in range(QT):
                es = []
                M = sc_pool.tile([128, GSZ], bfl6, tag="m")
                for g in range(NGROUP):
                    pg = psum_pool.tile([128, GSZ], fp32, tag="pg")
                    for band in range(GROUP):
                        bp = 32 * band
                        j = (g * GROUP + band) * NCHUNK
                        nc.tensor.matmul(
                            pg[:, band * NCHUNK:(band + 1) * NCHUNK],
                            L[bp:bp + KP, t * 128:(t + 1) * 128],
                            Rf[bp:bp + KP, j:j + NCHUNK],
                            start=True, stop=True,
                            tile_position=(bp, 0),
                        )
                    # e = -d in bf16
                    e = e_pool.tile([128, GSZ], bfl6, tag="e")
                    nc.scalar.activation(e[:, :], pg[:, :], ACTF.Copy,
                                         scale=-1.0)
                    es.append(e)
                    # dir1: elementwise max-fold across groups (same queries)
                    if g == 1:
                        nc.vector.tensor_tensor(
                            out=M[:, :], in0=es[0][:, :], in1=es[1][:, :],
                            op=A.max)
                    elif g > 1:
                        nc.vector.tensor_tensor(
                            out=M[:, :], in0=e[:, :], in1=M[:, :], op=A.max)
                    # dir2: running col-max chain
                    rg = Racc[:, g * GSZ:(g + 1) * GSZ]
                    if t == 0:
                        nc.vector.tensor_scalar(
                            out=rg, in0=e[:, :], scalar1=-BIGF,
                            scalar2=None, op0=A.max)
                    elif g < DMACHAIN:
                        nc.gpsimd.dma_start(out=rg, in_=e[:, :],
                                            accum_op=A.max)
                    else:
                        nc.vector.tensor_tensor(
                            out=rg, in0=e[:, :], in1=rg, op=A.max)
                # dir1 tree: M [2048] -> W[:, t*64:(t+1)*64]
                w = GSZ // 2
                while w >= TREE_OUT:
                    dst = (W[:, t * TREE_OUT:(t + 1) * TREE_OUT]
                           if w == TREE_OUT else M[:, 0:w])
                    nc.vector.tensor_tensor(
                        out=dst, in0=M[:, 0:w], in1=M[:, w:2 * w], op=A.max)
                    w //= 2

            # batched tail: [128, QT, 64] -> [128, QT]
            Wv = W.rearrange("p (t n) -> p t n", t=QT)
            w = TREE_OUT // 2
            while w >= 1:
                nc.vector.tensor_tensor(
                    out=Wv[:, :, 0:w], in0=Wv[:, :, 0:w],
                    in1=Wv[:, :, w:2 * w], op=A.max)
                w //= 2
            # W holds max(-d) per qtile; dist1 per query = max(-m, 0)
            nc.vector.tensor_scalar(
                out=qmin[:, :], in0=Wv[:, :, 0], scalar1=-1.0, scalar2=0.0,
                op0=A.mult, op1=A.max)
            nc.vector.tensor_reduce(out=accsum[:, :], in_=qmin[:, :],
                                    axis=AX.X, op=A.add)
            nc.sync.dma_start(out=out1[:, :], in_=accsum[:, :])
            for c in range(NGROUP):
                nc.sync.dma_start(out=out2[:, c * GSZ:(c + 1) * GSZ],
                                  in_=Racc[:, c * GSZ:(c + 1) * GSZ])
    nc.finalize()
    return nc


def _split_bf16(x):
    hi = x.astype(bf16)
    lo = (x - hi.astype(np.float32)).astype(bf16)
    return hi, lo


def _pad_bands(rows):
    """[13, n] bf16 -> [128, n]: pad K to 32 with zeros, replicate 4x."""
    n = rows.shape[1]
    k32 = np.zeros((KP, n), dtype=bf16)
    k32[:K] = rows
    return np.concatenate([k32] * 4, axis=0)


def _make_lhsT(q):
    x = np.ascontiguousarray(q.T).astype(np.float32)
    x2 = np.sum(q * q, axis=-1, dtype=np.float32)
    xh, xl = _split_bf16(x)
    x2h, x2l = _split_bf16(x2)
    ones = np.ones_like(x2, dtype=bf16)
    rows = np.concatenate([xh, xh, xl, x2h[None], x2l[None],
                           ones[None], ones[None]], axis=0)
    return _pad_bands(rows)


def _make_rhs(d):
    y = np.ascontiguousarray((-2.0 * d.T)).astype(np.float32)
    y2 = np.sum(d * d, axis=-1, dtype=np.float32)
    yh, yl = _split_bf16(y)
    y2h, y2l = _split_bf16(y2)
    ones = np.ones_like(y2, dtype=bf16)
    rows = np.concatenate([yh, yl, yh, ones[None], ones[None],
                           y2h[None], y2l[None]], axis=0)
    return _pad_bands(rows)


def make_in_maps(points1, points2):
    p1 = np.asarray(points1, dtype=np.float32)
    p2 = np.asarray(points2, dtype=np.float32)
    in_maps = []
    for i in range(N_CORES):
        b, h = divmod(i, 2)
        in_maps.append({
            "la": _make_lhsT(p1[b, h * NQ:(h + 1) * NQ]),
            "ra": _make_rhs(p2[b]),
        })
    return in_maps


def combine(results):
    """Host-side gather: dir1 sums + max-combine of negated dir2 partials."""
    total = 0.0
    for i in range(N_CORES):
        total += float(results[i]["out1"].astype(np.float64).sum())
    for b in range(B):
        r = np.maximum(results[2 * b]["out2"], results[2 * b + 1]["out2"])
        colmax = r.astype(np.float32).max(axis=0)
        total += float(np.maximum(-colmax, 0.0).astype(np.float64).sum())
    return np.float32(total / N)


_CACHE = {}


def kernel(points1, points2):
    from concourse.bass_utils import run_bass_kernel_spmd

    if "nc" not in _CACHE:
        _CACHE["nc"] = build_bass()
    nc = _CACHE["nc"]
    in_maps = make_in_maps(points1, points2)
    res = run_bass_kernel_spmd(nc, in_maps, core_ids=list(range(N_CORES)))
    return combine(res.results)


# revision 18
# speedup vs baseline: 1.4462x; 1.0058x over previous
"""Chamfer distance kernel for 8 Trainium2 NeuronCores — v7 (shared matrix).

Each core computes ONE [4096 x 8192] block of the per-batch distance
matrix (p1-half rows x all p2 cols) instead of two directional passes:
half the matmul + PSUM-drain volume of v6.  Both reductions come from
the same block:
  drain: ScalarE ACT Copy with scale=-1 drains PSUM->SBUF bf16 (e = -d).
  dir1 (per-p1-row min): VectorE pool_max per group -> acc slot
       (max of -d == -min d); single-src pool runs in a fast DVE mode.
  dir2 (per-p2-col min): VectorE running elementwise-max chain over
       qtiles into Racc [128, 8192] (still negated); host max-combines
       the two cores of each batch and reduces over the partition axis
       (the "min-combining" step of the sharding hint).
"""

import os
import numpy as np
import ml_dtypes

bf16 = ml_dtypes.bfloat16

B = 4
N = 8192            # points per cloud
NQ = N // 2         # p1 rows handled per core
K = 13              # real contraction rows (padded to 32 per band)
KP = 32
QT = NQ // 128      # query tiles per core (32)
NCHUNK = 512        # db points per matmul (one PSUM bank stripe)
GROUP = 4           # chunks per PSUM group tile
GSZ = GROUP * NCHUNK            # 2048
NGROUP = N // GSZ   # 4 groups per qtile
N_CORES = 8
BIGF = 1.0e30

DMACHAIN = int(os.environ.get("V7_DMACHAIN", "0"))  # groups chained via DMA CCE
TREE_OUT = int(os.environ.get("V7_TREEOUT", "1024"))


def build_bass():
    import concourse.bacc as bacc
    import concourse.mybir as mybir
    from concourse.tile import TileContext

    fp32 = mybir.dt.float32
    bfl6 = mybir.dt.bfloat16
    A = mybir.AluOpType
    AX = mybir.AxisListType
    ACTF = mybir.ActivationFunctionType

    nc = bacc.Bacc()

    la = nc.declare_dram_parameter("la", [128, NQ], bfl6, isOutput=False)
    ra = nc.declare_dram_parameter("ra", [128, N], bfl6, isOutput=False)
    out1 = nc.declare_dram_parameter("out1", [128, 1], fp32, isOutput=True)
    out2 = nc.declare_dram_parameter("out2", [128, N], bfl6, isOutput=True)

    with TileContext(nc) as tc:
        with (
            tc.tile_pool(name="ops", bufs=1) as ops_pool,
            tc.tile_pool(name="psum", bufs=2, space="PSUM") as psum_pool,
            tc.tile_pool(name="exit", bufs=3) as e_pool,
            tc.tile_pool(name="scrap", bufs=2) as sc_pool,
        ):
            L = ops_pool.tile([128, NQ], bfl6, tag="L", name="L")
            Rf = ops_pool.tile([128, N], bfl6, tag="Rf", name="Rf")
            Racc = ops_pool.tile([128, N], bfl6, tag="Racc", name="Racc")
            W = ops_pool.tile([128, QT * TREE_OUT], bfl6, tag="W")
            qmin = ops_pool.tile([128, QT], fp32, tag="qmin")
            accsum = ops_pool.tile([128, 1], fp32, tag="accsum")

            nc.scalar.dma_start(out=L[:, :], in_=la[:, :])
            for c in range(NGROUP):
                nc.sync.dma_start(out=Rf[:, c * GSZ:(c + 1) * GSZ],
                                  in_=ra[:, c * GSZ:(c + 1) * GSZ])
            for bp in (0, 32, 64, 96):
                nc.tensor.ldweights(L[bp:bp + KP, 0:128],
                                    tile_position=(bp, 0))
                nc.tensor.ldweights(Rf[bp:bp + KP, 0:128],
                                    tile_position=(bp, 0))

            for t in range(QT):
                M = sc_pool.tile([128, GSZ], bfl6, tag="m")
                e4 = e_pool.tile([128, N], bfl6, tag="e4")
                for g in range(NGROUP):
                    pg = psum_pool.tile([128, GSZ], fp32, tag="pg")
                    for band in range(GROUP):
                        bp = 32 * band
                        j = (g * GROUP + band) * NCHUNK
                        nc.tensor.matmul(
                            pg[:, band * NCHUNK:(band + 1) * NCHUNK],
                            L[bp:bp + KP, t * 128:(t + 1) * 128],
                            Rf[bp:bp + KP, j:j + NCHUNK],
                            start=True, stop=True,
                            tile_position=(bp, 0),
                        )
                    # e = -d in bf16 (quarter slice of the qtile-wide tile)
                    e = e4[:, g * GSZ:(g + 1) * GSZ]
                    nc.scalar.activation(e, pg[:, :], ACTF.Copy, scale=-1.0)
                    # dir1: elementwise max-fold across groups (same queries)
                    if g == 1:
                        nc.vector.tensor_tensor(
                            out=M[:, :], in0=e4[:, 0:GSZ], in1=e, op=A.max)
                    elif g > 1:
                        nc.vector.tensor_tensor(
                            out=M[:, :], in0=e, in1=M[:, :], op=A.max)
                # dir2: running col-max chain, full qtile width
                if t == 0:
                    nc.vector.tensor_scalar(
                        out=Racc[:, :], in0=e4[:, :], scalar1=-BIGF,
                        scalar2=None, op0=A.max)
                else:
                    nc.vector.tensor_tensor(
                        out=Racc[:, :], in0=e4[:, :], in1=Racc[:, :],
                        op=A.max)
                # dir1 tree: M [2048] -> W[:, t*1024:(t+1)*1024]
                w = GSZ // 2
                while w >= TREE_OUT:
                    dst = (W[:, t * TREE_OUT:(t + 1) * TREE_OUT]
                           if w == TREE_OUT else M[:, 0:w])
                    nc.vector.tensor_tensor(
                        out=dst, in0=M[:, 0:w], in1=M[:, w:2 * w], op=A.max)
                    w //= 2

            # batched tail: [128, QT, 64] -> [128, QT]
            Wv = W.rearrange("p (t n) -> p t n", t=QT)
            w = TREE_OUT // 2
            while w >= 1:
                nc.vector.tensor_tensor(
                    out=Wv[:, :, 0:w], in0=Wv[:, :, 0:w],
                    in1=Wv[:, :, w:2 * w], op=A.max)
                w //= 2
            # W holds max(-d) per qtile; dist1 per query = max(-m, 0)
            nc.vector.tensor_scalar(
                out=qmin[:, :], in0=Wv[:, :, 0], scalar1=-1.0, scalar2=0.0,
                op0=A.mult, op1=A.max)
            nc.vector.tensor_reduce(out=accsum[:, :], in_=qmin[:, :],
                                    axis=AX.X, op=A.add)
            nc.sync.dma_start(out=out1[:, :], in_=accsum[:, :])
            for c in range(NGROUP):
                nc.sync.dma_start(out=out2[:, c * GSZ:(c + 1) * GSZ],
                                  in_=Racc[:, c * GSZ:(c + 1) * GSZ])
    nc.finalize()
    return nc


def _split_bf16(x):
    hi = x.astype(bf16)
    lo = (x - hi.astype(np.float32)).astype(bf16)
    return hi, lo


def _pad_bands(rows):
    """[13, n] bf16 -> [128, n]: pad K to 32 with zeros, replicate 4x."""
    n = rows.shape[1]
    k32 = np.zeros((KP, n), dtype=bf16)
    k32[:K] = rows
    return np.concatenate([k32] * 4, axis=0)


def _make_lhsT(q):
    x = np.ascontiguousarray(q.T).astype(np.float32)
    x2 = np.sum(q * q, axis=-1, dtype=np.float32)
    xh, xl = _split_bf16(x)
    x2h, x2l = _split_bf16(x2)
    ones = np.ones_like(x2, dtype=bf16)
    rows = np.concatenate([xh, xh, xl, x2h[None], x2l[None],
                           ones[None], ones[None]], axis=0)
    return _pad_bands(rows)


def _make_rhs(d):
    y = np.ascontiguousarray((-2.0 * d.T)).astype(np.float32)
    y2 = np.sum(d * d, axis=-1, dtype=np.float32)
    yh, yl = _split_bf16(y)
    y2h, y2l = _split_bf16(y2)
    ones = np.ones_like(y2, dtype=bf16)
    rows = np.concatenate([yh, yl, yh, ones[None], ones[None],
                           y2h[None], y2l[None]], axis=0)
    return _pad_bands(rows)


def make_in_maps(points1, points2):
    p1 = np.asarray(points1, dtype=np.float32)
    p2 = np.asarray(points2, dtype=np.float32)
    in_maps = []
    for i in range(N_CORES):
        b, h = divmod(i, 2)
        in_maps.append({
            "la": _make_lhsT(p1[b, h * NQ:(h + 1) * NQ]),
            "ra": _make_rhs(p2[b]),
        })
    return in_maps


def combine(results):
    """Host-side gather: dir1 sums + max-combine of negated dir2 partials."""
    total = 0.0
    for i in range(N_CORES):
        total += float(results[i]["out1"].astype(np.float64).sum())
    for b in range(B):
        r = np.maximum(results[2 * b]["out2"], results[2 * b + 1]["out2"])
        colmax = r.astype(np.float32).max(axis=0)
        total += float(np.maximum(-colmax, 0.0).astype(np.float64).sum())
    return np.float32(total / N)


_CACHE = {}


def kernel(points1, points2):
    from concourse.bass_utils import run_bass_kernel_spmd

    if "nc" not in _CACHE:
        _CACHE["nc"] = build_bass()
    nc = _CACHE["nc"]
    in_maps = make_in_maps(points1, points2)
    res = run_bass_kernel_spmd(nc, in_maps, core_ids=list(range(N_CORES)))
    return combine(res.results)
